# revision 11
# baseline (speedup 1.0000x reference)
"""Multi-head attention (B=2, Q=K=2048, H=16, D=V=64) on 8 Trainium2 cores.

Sharding: batch x heads. Core c handles batch b = c//4 and heads
[4*(c%4), 4*(c%4)+4) -- 4 (b,h) "pairs" per core, no cross-core comm.

Key optimization vs the naive version: the key_mask zeroes ~half the
keys, and masked keys contribute exactly 0 to both the softmax numerator
and denominator (exp*mask). So the host compacts K/V to the unmasked
keys only (padded to a multiple of 128; pad keys have V''=0 and a zero
denominator column, so they contribute exactly nothing). This halves
the score matrix and with it the TensorE and ActE work. The host also
pre-transposes and pre-casts Q/K/V'' to bf16, so the device does no
input conversion at all.

Device algorithm per (b,h) pair (flash-style, no max subtraction:
scores/8 ~ N(0,1), so exp() is far from fp32 overflow; the reference's
max subtraction cancels in the softmax ratio up to a vanishing
eps*exp(-max) term):

  for each q-block (512 wide):
    for each k-chunk (128 compacted keys):
      S^T[k,q] = (K-chunk d,k)^T @ (Q^T d,q)   on TensorE (bf16, fp32 acc)
      E = exp(S/8)                              PSUM -> SBUF bf16
      acc[0:65, q] += V''^T @ E                 on TensorE (V'' = [V*m | m])
    normalize in O^T orientation: den = acc[64] + eps on ScalarE (Copy
    with bias), rec = 1/den on VectorE (reciprocal_approx_fast),
    partition-broadcast rec on GpSimd, O^T = acc * rec on VectorE.
    The host untransposes.

The exp is split between engines to keep the ScalarE off the critical
path: the first two chunk-groups of each block use the ScalarE exp; the
last group is computed on the VectorE with a Schraudolph-style fast
exp2 — i16 = round(s*(2^7*log2(e)/8) + (16256 - 7.25)) bit-cast as
bfloat16 gives exp(s/8) to ~1.8% RMS on ~25% of the weights (~0.6% on
the final output, measured 0.9% total vs the 2% gate).

Software pipelining: QK matmuls for exp-group g are emitted before the
PV matmuls of group g-1, so the TensorE never waits on the exp engines.
The normalize runs on ScalarE/VectorE/GpSimd, so the TensorE stream is
matmuls only. PSUM: 2x3-bank score windows + 2 acc banks = 8. Input
DMAs are split across the SP and ActE queues; outputs stream out
per-block in bf16.
"""

import math
import sys

import numpy as np

sys.path.insert(0, "/opt/trn_rl_repo")

import ml_dtypes

import concourse.bacc as bacc
import concourse.mybir as mybir
import concourse.tile as tile
from concourse.bass_utils import run_bass_kernel_spmd

BF16NP = ml_dtypes.bfloat16

N_CORES = 8
B, Q, K, H, D, V = 2, 2048, 2048, 16, 64, 64
PAIRS = 4            # (b,h) pairs per core
QBW = 512            # q-block width
QB = Q // QBW        # 4 q-blocks
G = 3                # k-chunks per exp group (3 PSUM banks per window)
EPS = 1e-10

F32 = mybir.dt.float32
BF16 = mybir.dt.bfloat16
I16 = mybir.dt.int16

# Schraudolph fast-exp constants (bf16 bit pattern via int16):
# i16 = s * (2^7 * log2(e) / 8) + (2^7 * 127 - C);  C tuned for min RMS.
SCH_A = 1.4426950408889634 * 128.0 / 8.0
SCH_B = 16256.0 - 7.25

_cached = {}
LAST_RESULTS = None


def _build_program(kc):
    """kc = number of 128-key chunks after mask compaction."""
    nc = bacc.Bacc("TRN2", target_bir_lowering=False, debug=False, num_devices=N_CORES)

    qT = nc.dram_tensor("qT", [PAIRS, D, Q], BF16, kind="ExternalInput").ap()
    kT = nc.dram_tensor("kT", [PAIRS, D, kc * 128], BF16, kind="ExternalInput").ap()
    v = nc.dram_tensor("v", [PAIRS, 128, kc, V + 1], BF16, kind="ExternalInput").ap()
    # output: [pair, blk, V, q-in-block] (O^T; host untransposes)
    o = nc.dram_tensor("o", [PAIRS, QB, V, QBW], BF16, kind="ExternalOutput").ap()

    with tile.TileContext(nc) as tc:
        with (
            tc.sbuf_pool(name="persist", bufs=1) as persist,
            tc.sbuf_pool(name="epool", bufs=3) as epool,
            tc.sbuf_pool(name="norm", bufs=2) as normp,
            tc.sbuf_pool(name="osbp", bufs=3) as osbp,
            tc.psum_pool(name="win", bufs=2) as winp,
            tc.psum_pool(name="acc", bufs=2) as accp,
        ):
            # ---------------- input DMAs (no device-side conversion) -------
            # pairs 0/1 stream in on the SP queue, pairs 2/3 on the ActE
            # queue; K before Q before V'' so pair 0's first matmuls can
            # start as early as possible.
            qTb, kTb, vppb = [None] * PAIRS, [None] * PAIRS, [None] * PAIRS
            for p in range(PAIRS):
                kTb[p] = persist.tile(
                    [D, kc * 128], BF16, tag=f"kTb{p}", name=f"kTb{p}"
                )
                qTb[p] = persist.tile([D, Q], BF16, tag=f"qTb{p}", name=f"qTb{p}")
                vppb[p] = persist.tile(
                    [128, kc, V + 1], BF16, tag=f"vpp{p}", name=f"vpp{p}"
                )
            for p in (0, 2, 1, 3):
                eng = nc.sync if p % 2 == 0 else nc.scalar
                eng.dma_start(out=kTb[p], in_=kT[p])
                eng.dma_start(out=qTb[p], in_=qT[p])
                eng.dma_start(out=vppb[p], in_=v[p])

            groups = [list(range(s, min(s + G, kc))) for s in range(0, kc, G)]

            def emit_mm2(p, acc, chunks, e):
                for i, c in enumerate(chunks):
                    nc.tensor.matmul(
                        acc[:, :],
                        vppb[p][:, c, :],
                        e[:, i, :],
                        start=(c == 0),
                        stop=(c == kc - 1),
                    )

            def emit_norm(acc, p, blk):
                den = normp.tile([1, QBW], F32, tag="den")
                nc.scalar.activation(
                    out=den,
                    in_=acc[V : V + 1, :],
                    func=mybir.ActivationFunctionType.Copy,
                    bias=EPS,
                )
                rec = normp.tile([1, QBW], F32, tag="rec")
                nc.vector.reciprocal_approx_fast(out=rec, in_=den)
                bc = normp.tile([V, QBW], F32, tag="bc")
                nc.gpsimd.partition_broadcast(bc, rec)
                osb = osbp.tile([V, QBW], BF16, tag="osb")
                nc.vector.tensor_tensor(
                    out=osb, in0=acc[0:V, :], in1=bc, op=mybir.AluOpType.mult
                )
                nc.sync.dma_start(out=o[p, blk], in_=osb)

            # ---------------- main pipelined loops ----------------
            for p in range(PAIRS):
                for blk in range(QB):
                    q0 = blk * QBW
                    acc = accp.tile([V + 1, QBW], F32, tag="acc")
                    pending = None  # (chunks, e) awaiting PV matmul
                    for gi, chunks in enumerate(groups):
                        win = winp.tile([128, G, QBW], F32, tag="win")
                        for i, c in enumerate(chunks):
                            nc.tensor.matmul(
                                win[:, i, :],
                                kTb[p][:, c * 128 : (c + 1) * 128],
                                qTb[p][:, q0 : q0 + QBW],
                                start=True,
                                stop=True,
                            )
                        if pending is not None:
                            emit_mm2(p, acc, *pending)
                        n = len(chunks)
                        e = epool.tile([128, G, QBW], BF16, tag="e")
                        if gi < len(groups) - 1:
                            # exact exp on ScalarE
                            nc.scalar.activation(
                                out=e[:, :n, :],
                                in_=win[:, :n, :],
                                func=mybir.ActivationFunctionType.Exp,
                                scale=0.125,
                            )
                        else:
                            # Schraudolph fast exp on VectorE: int16 bits of
                            # the bf16 value exp(s/8)
                            nc.vector.tensor_scalar(
                                out=e[:, :n, :].bitcast(I16),
                                in0=win[:, :n, :],
                                scalar1=SCH_A,
                                scalar2=SCH_B,
                                op0=mybir.AluOpType.mult,
                                op1=mybir.AluOpType.add,
                            )
                        pending = (chunks, e)
                    emit_mm2(p, acc, *pending)
                    emit_norm(acc, p, blk)

    nc.compile()
    return nc


def _get_program(kc):
    if kc not in _cached:
        _cached[kc] = _build_program(kc)
    return _cached[kc]


def _shard_inputs(queries, keys, values, key_mask):
    q = np.asarray(queries, dtype=np.float32)
    k = np.asarray(keys, dtype=np.float32)
    v = np.asarray(values, dtype=np.float32)
    m = np.asarray(key_mask)

    idx = [np.nonzero(m[b])[0] for b in range(B)]
    keff = max(len(ix) for ix in idx)
    kc = max(1, math.ceil(keff / 128))
    kp = kc * 128

    # [B, S, H, D] -> [B, H, D, S], bf16
    qT = np.ascontiguousarray(q.transpose(0, 2, 3, 1)).astype(BF16NP)

    # compacted K^T and V'' = [V*m | m], zero-padded to kp keys
    kT = np.zeros((B, H, D, kp), dtype=np.float32)
    vpp = np.zeros((B, H, kp, V + 1), dtype=np.float32)
    for b in range(B):
        ix = idx[b]
        n = len(ix)
        if n == 0:
            continue
        mb = m[b, ix].astype(np.float32)
        kT[b, :, :, :n] = k[b, ix].transpose(1, 2, 0)
        vpp[b, :, :n, :V] = (v[b, ix] * mb[:, None, None]).transpose(1, 0, 2)
        vpp[b, :, :n, V] = mb[None, :]
    kTb = kT.astype(BF16NP)
    # [B, H, kp, V+1] -> [B, H, 128(r), kc, V+1]  (key kk = c*128 + r)
    vppb = np.ascontiguousarray(
        vpp.reshape(B, H, kc, 128, V + 1).transpose(0, 1, 3, 2, 4)
    ).astype(BF16NP)

    in_maps = []
    for core in range(N_CORES):
        b, h0 = core // 4, (core % 4) * 4
        in_maps.append(
            {
                "qT": np.ascontiguousarray(qT[b, h0 : h0 + 4]),
                "kT": np.ascontiguousarray(kTb[b, h0 : h0 + 4]),
                "v": np.ascontiguousarray(vppb[b, h0 : h0 + 4]),
            }
        )
    return in_maps, kc


def kernel(queries, keys, values, key_mask):
    global LAST_RESULTS
    in_maps, kc = _shard_inputs(queries, keys, values, key_mask)
    nc = _get_program(kc)
    res = run_bass_kernel_spmd(nc, in_maps, list(range(N_CORES)))
    LAST_RESULTS = res

    out = np.empty((B, Q, H * V), dtype=np.float32)
    for core in range(N_CORES):
        b, h0 = core // 4, (core % 4) * 4
        # [PAIRS, QB, V, QBW] -> [PAIRS, Q, V]
        oc = (
            res.results[core]["o"]
            .astype(np.float32)
            .transpose(0, 1, 3, 2)
            .reshape(PAIRS, Q, V)
        )
        for p in range(PAIRS):
            h = h0 + p
            out[b, :, h * V : (h + 1) * V] = oc[p]
    return out


# revision 14
# speedup vs baseline: 1.0179x; 1.0179x over previous
"""Multi-head attention (B=2, Q=K=2048, H=16, D=V=64) on 8 Trainium2 cores.

Sharding: batch x heads. Core c handles batch b = c//4 and heads
[4*(c%4), 4*(c%4)+4) -- 4 (b,h) "pairs" per core, no cross-core comm.

Key optimization vs the naive version: the key_mask zeroes ~half the
keys, and masked keys contribute exactly 0 to both the softmax numerator
and denominator (exp*mask). So the host compacts K/V to the unmasked
keys only (padded to a multiple of 128; pad keys have V''=0 and a zero
denominator column, so they contribute exactly nothing). This halves
the score matrix and with it the TensorE and ActE work. The host also
pre-transposes and pre-casts Q/K/V'' to bf16, so the device does no
input conversion at all.

Device algorithm per (b,h) pair (flash-style, no max subtraction:
scores/8 ~ N(0,1), so exp() is far from fp32 overflow; the reference's
max subtraction cancels in the softmax ratio up to a vanishing
eps*exp(-max) term):

  for each q-block (512 wide):
    for each k-chunk (128 compacted keys):
      S^T[k,q] = (K-chunk d,k)^T @ (Q^T d,q)   on TensorE (bf16, fp32 acc)
      E = exp(S/8)                              PSUM -> SBUF bf16
      acc[0:65, q] += V''^T @ E                 on TensorE (V'' = [V*m | m])
    normalize in O^T orientation: den = acc[64] + eps on ScalarE (Copy
    with bias), rec = 1/den on VectorE (reciprocal_approx_fast),
    partition-broadcast rec on GpSimd, O^T = acc * rec on VectorE.
    The host untransposes.

The exp is split between engines to keep the ScalarE off the critical
path: the first two chunk-groups of each block use the ScalarE exp; the
last group is computed on the VectorE with a Schraudolph-style fast
exp2 — i16 = round(s*(2^7*log2(e)/8) + (16256 - 7.25)) bit-cast as
bfloat16 gives exp(s/8) to ~1.8% RMS on ~25% of the weights (~0.6% on
the final output, measured 0.9% total vs the 2% gate).

Software pipelining: QK matmuls for exp-group g are emitted before the
PV matmuls of group g-1, so the TensorE never waits on the exp engines.
The normalize runs on ScalarE/VectorE/GpSimd, so the TensorE stream is
matmuls only. PSUM: 2x3-bank score windows + 2 acc banks = 8. Input
DMAs are split across the SP and ActE queues; outputs stream out
per-block in bf16.
"""

import math
import sys

import numpy as np

sys.path.insert(0, "/opt/trn_rl_repo")

import ml_dtypes

import concourse.bacc as bacc
import concourse.mybir as mybir
import concourse.tile as tile
from concourse.bass_utils import run_bass_kernel_spmd

BF16NP = ml_dtypes.bfloat16

N_CORES = 8
B, Q, K, H, D, V = 2, 2048, 2048, 16, 64, 64
PAIRS = 4            # (b,h) pairs per core
QBW = 512            # q-block width
QB = Q // QBW        # 4 q-blocks
G = 3                # k-chunks per exp group (3 PSUM banks per window)
EPS = 1e-10

F32 = mybir.dt.float32
BF16 = mybir.dt.bfloat16
I16 = mybir.dt.int16

# Schraudolph fast-exp constants (bf16 bit pattern via int16):
# i16 = s * (2^7 * log2(e) / 8) + (2^7 * 127 - C);  C tuned for min RMS.
SCH_A = 1.4426950408889634 * 128.0 / 8.0
SCH_B = 16256.0 - 7.25

_cached = {}
LAST_RESULTS = None


def _build_program(kc):
    """kc = number of 128-key chunks after mask compaction."""
    nc = bacc.Bacc("TRN2", target_bir_lowering=False, debug=False, num_devices=N_CORES)

    qT = nc.dram_tensor("qT", [PAIRS, D, Q], BF16, kind="ExternalInput").ap()
    kT = nc.dram_tensor("kT", [PAIRS, D, kc * 128], BF16, kind="ExternalInput").ap()
    v = nc.dram_tensor("v", [PAIRS, 128, kc, V + 1], BF16, kind="ExternalInput").ap()
    # output: [pair, blk, V, q-in-block] (O^T; host untransposes)
    o = nc.dram_tensor("o", [PAIRS, QB, V, QBW], BF16, kind="ExternalOutput").ap()

    with tile.TileContext(nc) as tc:
        with (
            tc.sbuf_pool(name="persist", bufs=1) as persist,
            tc.sbuf_pool(name="epool", bufs=3) as epool,
            tc.sbuf_pool(name="norm", bufs=2) as normp,
            tc.sbuf_pool(name="osbp", bufs=3) as osbp,
            tc.psum_pool(name="win", bufs=2) as winp,
            tc.psum_pool(name="acc", bufs=2) as accp,
        ):
            # ---------------- input DMAs (no device-side conversion) -------
            # pairs 0/1 stream in on the SP queue, pairs 2/3 on the ActE
            # queue; K before Q before V'' so pair 0's first matmuls can
            # start as early as possible.
            qTb, kTb, vppb = [None] * PAIRS, [None] * PAIRS, [None] * PAIRS
            for p in range(PAIRS):
                kTb[p] = persist.tile(
                    [D, kc * 128], BF16, tag=f"kTb{p}", name=f"kTb{p}"
                )
                qTb[p] = persist.tile([D, Q], BF16, tag=f"qTb{p}", name=f"qTb{p}")
                vppb[p] = persist.tile(
                    [128, kc, V + 1], BF16, tag=f"vpp{p}", name=f"vpp{p}"
                )
            # pair 0's first q-block arrives separately so the first matmul
            # can start before the rest of Q streams in
            nc.sync.dma_start(out=kTb[0], in_=kT[0])
            nc.sync.dma_start(out=qTb[0][:, 0:QBW], in_=qT[0][:, 0:QBW])
            nc.sync.dma_start(out=vppb[0], in_=v[0])
            nc.sync.dma_start(out=qTb[0][:, QBW:Q], in_=qT[0][:, QBW:Q])
            for p in (1, 2, 3):
                eng = nc.scalar if p % 2 == 1 else nc.sync
                eng.dma_start(out=kTb[p], in_=kT[p])
                eng.dma_start(out=qTb[p], in_=qT[p])
                eng.dma_start(out=vppb[p], in_=v[p])

            groups = [list(range(s, min(s + G, kc))) for s in range(0, kc, G)]

            def emit_mm2(p, acc, chunks, e):
                for i, c in enumerate(chunks):
                    nc.tensor.matmul(
                        acc[:, :],
                        vppb[p][:, c, :],
                        e[:, i, :],
                        start=(c == 0),
                        stop=(c == kc - 1),
                    )

            def emit_norm(acc, p, blk):
                den = normp.tile([1, QBW], F32, tag="den")
                nc.scalar.activation(
                    out=den,
                    in_=acc[V : V + 1, :],
                    func=mybir.ActivationFunctionType.Copy,
                    bias=EPS,
                )
                rec = normp.tile([1, QBW], F32, tag="rec")
                nc.vector.reciprocal_approx_fast(out=rec, in_=den)
                bc = normp.tile([V, QBW], F32, tag="bc")
                nc.gpsimd.partition_broadcast(bc, rec)
                osb = osbp.tile([V, QBW], BF16, tag="osb")
                nc.vector.tensor_tensor(
                    out=osb, in0=acc[0:V, :], in1=bc, op=mybir.AluOpType.mult
                )
                nc.sync.dma_start(out=o[p, blk], in_=osb)

            # ---------------- main pipelined loops ----------------
            # norm(b-1) is emitted at the top of block b, so by the time each
            # of its ops reaches the head of its engine queue the deps are
            # long satisfied -- no engine blocks another through queue order.
            deferred = None  # (acc, p, blk) awaiting normalize
            for p in range(PAIRS):
                for blk in range(QB):
                    q0 = blk * QBW
                    acc = accp.tile([V + 1, QBW], F32, tag="acc")
                    if deferred is not None:
                        emit_norm(*deferred)
                        deferred = None
                    pending = None  # (chunks, e) awaiting PV matmul
                    for gi, chunks in enumerate(groups):
                        win = winp.tile([128, G, QBW], F32, tag="win")
                        for i, c in enumerate(chunks):
                            nc.tensor.matmul(
                                win[:, i, :],
                                kTb[p][:, c * 128 : (c + 1) * 128],
                                qTb[p][:, q0 : q0 + QBW],
                                start=True,
                                stop=True,
                            )
                        if pending is not None:
                            emit_mm2(p, acc, *pending)
                        n = len(chunks)
                        e = epool.tile([128, G, QBW], BF16, tag="e")
                        if gi < len(groups) - 1:
                            # exact exp on ScalarE
                            nc.scalar.activation(
                                out=e[:, :n, :],
                                in_=win[:, :n, :],
                                func=mybir.ActivationFunctionType.Exp,
                                scale=0.125,
                            )
                        else:
                            # Schraudolph fast exp on VectorE: int16 bits of
                            # the bf16 value exp(s/8)
                            nc.vector.tensor_scalar(
                                out=e[:, :n, :].bitcast(I16),
                                in0=win[:, :n, :],
                                scalar1=SCH_A,
                                scalar2=SCH_B,
                                op0=mybir.AluOpType.mult,
                                op1=mybir.AluOpType.add,
                            )
                        pending = (chunks, e)
                    emit_mm2(p, acc, *pending)
                    deferred = (acc, p, blk)
            emit_norm(*deferred)

    nc.compile()
    return nc


def _get_program(kc):
    if kc not in _cached:
        _cached[kc] = _build_program(kc)
    return _cached[kc]


def _shard_inputs(queries, keys, values, key_mask):
    q = np.asarray(queries, dtype=np.float32)
    k = np.asarray(keys, dtype=np.float32)
    v = np.asarray(values, dtype=np.float32)
    m = np.asarray(key_mask)

    idx = [np.nonzero(m[b])[0] for b in range(B)]
    keff = max(len(ix) for ix in idx)
    kc = max(1, math.ceil(keff / 128))
    kp = kc * 128

    # [B, S, H, D] -> [B, H, D, S], bf16
    qT = np.ascontiguousarray(q.transpose(0, 2, 3, 1)).astype(BF16NP)

    # compacted K^T and V'' = [V*m | m], zero-padded to kp keys
    kT = np.zeros((B, H, D, kp), dtype=np.float32)
    vpp = np.zeros((B, H, kp, V + 1), dtype=np.float32)
    for b in range(B):
        ix = idx[b]
        n = len(ix)
        if n == 0:
            continue
        mb = m[b, ix].astype(np.float32)
        kT[b, :, :, :n] = k[b, ix].transpose(1, 2, 0)
        vpp[b, :, :n, :V] = (v[b, ix] * mb[:, None, None]).transpose(1, 0, 2)
        vpp[b, :, :n, V] = mb[None, :]
    kTb = kT.astype(BF16NP)
    # [B, H, kp, V+1] -> [B, H, 128(r), kc, V+1]  (key kk = c*128 + r)
    vppb = np.ascontiguousarray(
        vpp.reshape(B, H, kc, 128, V + 1).transpose(0, 1, 3, 2, 4)
    ).astype(BF16NP)

    in_maps = []
    for core in range(N_CORES):
        b, h0 = core // 4, (core % 4) * 4
        in_maps.append(
            {
                "qT": np.ascontiguousarray(qT[b, h0 : h0 + 4]),
                "kT": np.ascontiguousarray(kTb[b, h0 : h0 + 4]),
                "v": np.ascontiguousarray(vppb[b, h0 : h0 + 4]),
            }
        )
    return in_maps, kc


def kernel(queries, keys, values, key_mask):
    global LAST_RESULTS
    in_maps, kc = _shard_inputs(queries, keys, values, key_mask)
    nc = _get_program(kc)
    res = run_bass_kernel_spmd(nc, in_maps, list(range(N_CORES)))
    LAST_RESULTS = res

    out = np.empty((B, Q, H * V), dtype=np.float32)
    for core in range(N_CORES):
        b, h0 = core // 4, (core % 4) * 4
        # [PAIRS, QB, V, QBW] -> [PAIRS, Q, V]
        oc = (
            res.results[core]["o"]
            .astype(np.float32)
            .transpose(0, 1, 3, 2)
            .reshape(PAIRS, Q, V)
        )
        for p in range(PAIRS):
            h = h0 + p
            out[b, :, h * V : (h + 1) * V] = oc[p]
    return out


# revision 18
# speedup vs baseline: 1.4336x; 1.4084x over previous
"""Multi-head attention (B=2, Q=K=2048, H=16, D=V=64) on 8 Trainium2 cores.

Sharding: batch x heads. Core c handles batch b = c//4 and heads
[4*(c%4), 4*(c%4)+4) -- 4 (b,h) "pairs" per core, no cross-core comm.

Key optimization vs the naive version: the key_mask zeroes ~half the
keys, and masked keys contribute exactly 0 to both the softmax numerator
and denominator (exp*mask). So the host compacts K/V to the unmasked
keys only (padded to a multiple of 128; pad keys have V''=0 and a zero
denominator column, so they contribute exactly nothing). This halves
the score matrix and with it the TensorE and ActE work. The host also
pre-transposes and pre-casts Q/K/V'' to bf16, so the device does no
input conversion at all.

Device algorithm per (b,h) pair (flash-style, no max subtraction:
scores/8 ~ N(0,1), so exp() is far from fp32 overflow; the reference's
max subtraction cancels in the softmax ratio up to a vanishing
eps*exp(-max) term):

  for each q-block (512 wide):
    for each k-chunk (128 compacted keys):
      S^T[k,q] = (K-chunk d,k)^T @ (Q^T d,q)   on TensorE (bf16, fp32 acc)
      E = exp(S/8)                              PSUM -> SBUF bf16
      acc[0:65, q] += V''^T @ E                 on TensorE (V'' = [V*m | m])
    normalize in O^T orientation: den = acc[64] + eps on ScalarE (Copy
    with bias), rec = 1/den on VectorE (reciprocal_approx_fast),
    partition-broadcast rec on GpSimd, O^T = acc * rec on VectorE.
    The host untransposes.

The exp is split between engines to keep the ScalarE off the critical
path: the first two chunk-groups of each block use the ScalarE exp; the
last group is computed on the VectorE with a Schraudolph-style fast
exp2 — i16 = round(s*(2^7*log2(e)/8) + (16256 - 7.25)) bit-cast as
bfloat16 gives exp(s/8) to ~1.8% RMS on ~25% of the weights (~0.6% on
the final output, measured 0.9% total vs the 2% gate).

Software pipelining: QK matmuls for exp-group g are emitted before the
PV matmuls of group g-1, so the TensorE never waits on the exp engines.
The normalize runs on ScalarE/VectorE/GpSimd, so the TensorE stream is
matmuls only. PSUM: 2x3-bank score windows + 2 acc banks = 8. Input
DMAs are split across the SP and ActE queues; outputs stream out
per-block in bf16.
"""

import math
import sys

import numpy as np

sys.path.insert(0, "/opt/trn_rl_repo")

import ml_dtypes

import concourse.bacc as bacc
import concourse.mybir as mybir
import concourse.tile as tile
from concourse.bass_utils import run_bass_kernel_spmd

BF16NP = ml_dtypes.bfloat16

N_CORES = 8
B, Q, K, H, D, V = 2, 2048, 2048, 16, 64, 64
PAIRS = 4            # (b,h) pairs per core
QBW = 512            # q-block width
QB = Q // QBW        # 4 q-blocks
G = 3                # k-chunks per exp group (3 PSUM banks per window)
EPS = 1e-10

F32 = mybir.dt.float32
BF16 = mybir.dt.bfloat16
I16 = mybir.dt.int16

# Schraudolph fast-exp constants (bf16 bit pattern via int16):
# i16 = s * (2^7 * log2(e) / 8) + (2^7 * 127 - C);  C tuned for min RMS.
SCH_A = 1.4426950408889634 * 128.0 / 8.0
SCH_B = 16256.0 - 7.25

_cached = {}
LAST_RESULTS = None


def _build_program(kc):
    """kc = number of 128-key chunks after mask compaction."""
    nc = bacc.Bacc("TRN2", target_bir_lowering=False, debug=False, num_devices=N_CORES)

    qT = nc.dram_tensor("qT", [PAIRS, D, Q], BF16, kind="ExternalInput").ap()
    kT = nc.dram_tensor("kT", [PAIRS, D, kc * 128], BF16, kind="ExternalInput").ap()
    v = nc.dram_tensor("v", [PAIRS, 128, kc, V + 1], BF16, kind="ExternalInput").ap()
    # output: [pair, blk, V, q-in-block] (O^T; host untransposes)
    o = nc.dram_tensor("o", [PAIRS, QB, V, QBW], BF16, kind="ExternalOutput").ap()

    with tile.TileContext(nc) as tc:
        with (
            tc.sbuf_pool(name="persist", bufs=1) as persist,
            tc.sbuf_pool(name="epool", bufs=3) as epool,
            tc.sbuf_pool(name="norm", bufs=2) as normp,
            tc.sbuf_pool(name="osbp", bufs=3) as osbp,
            tc.psum_pool(name="win", bufs=2) as winp,
            tc.psum_pool(name="acc", bufs=2) as accp,
        ):
            # ---------------- input DMAs (no device-side conversion) -------
            # pairs 0/1 stream in on the SP queue, pairs 2/3 on the ActE
            # queue; K before Q before V'' so pair 0's first matmuls can
            # start as early as possible.
            qTb, kTb, vppb = [None] * PAIRS, [None] * PAIRS, [None] * PAIRS
            for p in range(PAIRS):
                kTb[p] = persist.tile(
                    [D, kc * 128], BF16, tag=f"kTb{p}", name=f"kTb{p}"
                )
                qTb[p] = persist.tile([D, Q], BF16, tag=f"qTb{p}", name=f"qTb{p}")
                vppb[p] = persist.tile(
                    [128, kc, V + 1], BF16, tag=f"vpp{p}", name=f"vpp{p}"
                )
            # pair 0's first q-block arrives separately so the first matmul
            # can start before the rest of Q streams in
            nc.sync.dma_start(out=kTb[0], in_=kT[0])
            nc.sync.dma_start(out=qTb[0][:, 0:QBW], in_=qT[0][:, 0:QBW])
            nc.sync.dma_start(out=vppb[0], in_=v[0])
            nc.sync.dma_start(out=qTb[0][:, QBW:Q], in_=qT[0][:, QBW:Q])
            for p in (1, 2, 3):
                eng = nc.scalar if p % 2 == 1 else nc.sync
                eng.dma_start(out=kTb[p], in_=kT[p])
                eng.dma_start(out=qTb[p], in_=qT[p])
                eng.dma_start(out=vppb[p], in_=v[p])

            groups = [list(range(s, min(s + G, kc))) for s in range(0, kc, G)]
            # the last group runs on the VectorE (fast exp); give it a full
            # 3 chunks and the ScalarE the short group so the per-block
            # engine loads balance under the TensorE's
            if len(groups) >= 2 and len(groups[-1]) < len(groups[-2]):
                groups[-1], groups[-2] = groups[-2], groups[-1]

            def emit_mm2(p, acc, chunks, e):
                for i, c in enumerate(chunks):
                    nc.tensor.matmul(
                        acc[:, :],
                        vppb[p][:, c, :],
                        e[:, i, :],
                        start=(c == 0),
                        stop=(c == kc - 1),
                    )

            def emit_norm(acc, p, blk):
                # den must land in SBUF: the bit-twiddling custom-DVE
                # reciprocal misbehaves on PSUM reads
                den = normp.tile([1, QBW], F32, tag="den")
                nc.scalar.activation(
                    out=den,
                    in_=acc[V : V + 1, :],
                    func=mybir.ActivationFunctionType.Copy,
                    bias=EPS,
                )
                rec = normp.tile([1, QBW], F32, tag="rec")
                nc.vector.reciprocal_approx_fast(out=rec, in_=den)
                bc = normp.tile([V, QBW], F32, tag="bc")
                nc.gpsimd.partition_broadcast(bc, rec)
                osb = osbp.tile([V, QBW], BF16, tag="osb")
                nc.vector.tensor_tensor(
                    out=osb, in0=acc[0:V, :], in1=bc, op=mybir.AluOpType.mult
                )
                nc.sync.dma_start(out=o[p, blk], in_=osb)

            # ---------------- main pipelined loops ----------------
            # norm(b-1) is emitted at the top of block b, so by the time each
            # of its ops reaches the head of its engine queue the deps are
            # long satisfied -- no engine blocks another through queue order.
            deferred = None  # (acc, p, blk) awaiting normalize
            for p in range(PAIRS):
                for blk in range(QB):
                    q0 = blk * QBW
                    acc = accp.tile([V + 1, QBW], F32, tag="acc")
                    if deferred is not None:
                        emit_norm(*deferred)
                        deferred = None
                    pending = None  # (chunks, e) awaiting PV matmul
                    for gi, chunks in enumerate(groups):
                        win = winp.tile([128, G, QBW], F32, tag="win")
                        for i, c in enumerate(chunks):
                            nc.tensor.matmul(
                                win[:, i, :],
                                kTb[p][:, c * 128 : (c + 1) * 128],
                                qTb[p][:, q0 : q0 + QBW],
                                start=True,
                                stop=True,
                            )
                        if pending is not None:
                            emit_mm2(p, acc, *pending)
                        n = len(chunks)
                        e = epool.tile([128, G, QBW], BF16, tag="e")
                        if gi < len(groups) - 1:
                            # exact exp on ScalarE
                            nc.scalar.activation(
                                out=e[:, :n, :],
                                in_=win[:, :n, :],
                                func=mybir.ActivationFunctionType.Exp,
                                scale=0.125,
                            )
                        else:
                            # Schraudolph fast exp on VectorE: int16 bits of
                            # the bf16 value exp(s/8)
                            nc.vector.tensor_scalar(
                                out=e[:, :n, :].bitcast(I16),
                                in0=win[:, :n, :],
                                scalar1=SCH_A,
                                scalar2=SCH_B,
                                op0=mybir.AluOpType.mult,
                                op1=mybir.AluOpType.add,
                            )
                        pending = (chunks, e)
                    emit_mm2(p, acc, *pending)
                    deferred = (acc, p, blk)
            emit_norm(*deferred)

    nc.compile()
    return nc


def _get_program(kc):
    if kc not in _cached:
        _cached[kc] = _build_program(kc)
    return _cached[kc]


def _shard_inputs(queries, keys, values, key_mask):
    q = np.asarray(queries, dtype=np.float32)
    k = np.asarray(keys, dtype=np.float32)
    v = np.asarray(values, dtype=np.float32)
    m = np.asarray(key_mask)

    idx = [np.nonzero(m[b])[0] for b in range(B)]
    keff = max(len(ix) for ix in idx)
    kc = max(1, math.ceil(keff / 128))
    kp = kc * 128

    # [B, S, H, D] -> [B, H, D, S], bf16
    qT = np.ascontiguousarray(q.transpose(0, 2, 3, 1)).astype(BF16NP)

    # compacted K^T and V'' = [V*m | m], zero-padded to kp keys
    kT = np.zeros((B, H, D, kp), dtype=np.float32)
    vpp = np.zeros((B, H, kp, V + 1), dtype=np.float32)
    for b in range(B):
        ix = idx[b]
        n = len(ix)
        if n == 0:
            continue
        mb = m[b, ix].astype(np.float32)
        kT[b, :, :, :n] = k[b, ix].transpose(1, 2, 0)
        vpp[b, :, :n, :V] = (v[b, ix] * mb[:, None, None]).transpose(1, 0, 2)
        vpp[b, :, :n, V] = mb[None, :]
    kTb = kT.astype(BF16NP)
    # [B, H, kp, V+1] -> [B, H, 128(r), kc, V+1]  (key kk = c*128 + r)
    vppb = np.ascontiguousarray(
        vpp.reshape(B, H, kc, 128, V + 1).transpose(0, 1, 3, 2, 4)
    ).astype(BF16NP)

    in_maps = []
    for core in range(N_CORES):
        b, h0 = core // 4, (core % 4) * 4
        in_maps.append(
            {
                "qT": np.ascontiguousarray(qT[b, h0 : h0 + 4]),
                "kT": np.ascontiguousarray(kTb[b, h0 : h0 + 4]),
                "v": np.ascontiguousarray(vppb[b, h0 : h0 + 4]),
            }
        )
    return in_maps, kc


def kernel(queries, keys, values, key_mask):
    global LAST_RESULTS
    in_maps, kc = _shard_inputs(queries, keys, values, key_mask)
    nc = _get_program(kc)
    res = run_bass_kernel_spmd(nc, in_maps, list(range(N_CORES)))
    LAST_RESULTS = res

    out = np.empty((B, Q, H * V), dtype=np.float32)
    fully_masked = [not np.any(np.asarray(key_mask)[b]) for b in range(B)]
    for core in range(N_CORES):
        b, h0 = core // 4, (core % 4) * 4
        if fully_masked[b]:
            out[b] = 0.0
            continue
        # [PAIRS, QB, V, QBW] -> [PAIRS, Q, V]
        oc = (
            res.results[core]["o"]
            .astype(np.float32)
            .transpose(0, 1, 3, 2)
            .reshape(PAIRS, Q, V)
        )
        for p in range(PAIRS):
            h = h0 + p
            out[b, :, h * V : (h + 1) * V] = oc[p]
    return out
